# revision 21
# baseline (speedup 1.0000x reference)
"""Trainium2 Bass kernel for nn_FRAMES_VisionTransformer_28166395527587.

The reference computation (drop CLS token -> 1D nearest resize 768->729 ->
reverse-patching reshape to (144,126,126) -> 3D nearest resize to (64,64,64))
is a pure gather with compile-time-constant index maps:

    out[b, 0, z, y, x] = hs[b, 1 + 196*(z//4) + 14*r(y) + p(x),
                            f[81*d0(z) + 9*d1(y) + d2(x)]]

with  d0(z) = [0,2,4,6][z%4],          i(z) = z//4
      c(y)  = floor(63y/32) = 9*r + d1  (0, then odds 1..63, evens 64..124)
      c(x)  = floor(63x/32) = 9*p + d2  (same map)
      f[j]  = floor32(j*768/729)        (float32 floor, matching jax)

Sharding: pure data parallel, 8 batch samples per core.  The CLS token is
stripped host-side so the (sample, couple) block stride is uniform; on each
core the 128 SBUF partitions then hold the 128 (sample, couple) blocks and
every DMA spans all 128 partitions (all 16 SDMA engines).  The gather runs
as a short sequence of strided on-chip copies shared by all partitions.

Work is split into 16 sub-rounds (q, h, token-row half); the two halves of
each round load through different DMA queues (HWDGE + SWDGE) concurrently.
"""

import numpy as np

# ---------------------------------------------------------------- constants
B_FULL = 64
N_CORES = 8
B_CORE = B_FULL // N_CORES  # 8 samples per core


def _nearest_f32(out_size, in_size):
    """float32-exact emulation of the reference's jnp _nearest_idx.

    jax computes floor(arange(out) * (in/out)) in float32; at j=486 the
    product rounds to 511.999... so floor gives 511, not the exact 512."""
    ratio = np.float32(in_size / out_size)
    j = np.arange(out_size, dtype=np.int32).astype(np.float32)
    return np.floor((j * ratio).astype(np.float32)).astype(np.int64)


_f = _nearest_f32(729, 768)  # feature resize map
_c = _nearest_f32(64, 126)  # y/x resize map (= 9*r + d1)

DZ = [0, 2, 4, 6]  # d0 values for z%4
LOS = [int(_f[81 * d0]) for d0 in DZ]  # [0, 170, 341, 511]
UW = 96  # features per token: 384 B = 3x128 B aligned descriptors

# token-row split of each (q, h) round: rows 0-2 -> yl 0-13, rows 3-6 -> 14-31
SUBS = [(0, 3, 0, 14), (3, 4, 14, 18)]  # (rt0, nrt, yl0, nyl)


def _feat_runs(q):
    """Contiguous runs of the 81-feature selection for d0-slice q.

    Returns [(j0, n, u0)]: M[:, j0:j0+n] = L[:, u0:u0+n]."""
    g = _f[81 * DZ[q] + np.arange(81)] - LOS[q]
    runs, start = [], 0
    for k in range(1, 81):
        if g[k] != g[k - 1] + 1:
            runs.append((start, k - start, int(g[start])))
            start = k
    runs.append((start, 81 - start, int(g[start])))
    return runs


FEAT_RUNS = [_feat_runs(q) for q in range(4)]


def _x_runs():
    """x-gather runs: [(p, x0, nx, d20)] with d2 = d20+2k, x = x0+k."""
    runs, x = [], 0
    while x < 64:
        p, d20 = int(_c[x]) // 9, int(_c[x]) % 9
        n = 1
        while x + n < 64 and _c[x + n] == _c[x] + 2 * n and _c[x + n] // 9 == p:
            n += 1
        runs.append((p, x, n, d20))
        x += n
    return runs


X_RUNS = _x_runs()


def _y_runs(rt0, nrt):
    """y-gather runs for rows [rt0, rt0+nrt): [(yl0, n, c0)] with
    O[yl0+k] = X[c0+2k] in the sub-round's local c coordinates.

    The local map cl(yl) = c(32h+yl) - 63h is identical for both h."""
    cl = [int(_c[yl]) for yl in range(32)]  # h=0 local map: [0,1,3,...,61]
    lo, hi = 9 * rt0, 9 * (rt0 + nrt)
    yls = [yl for yl in range(32) if lo <= cl[yl] < hi]
    runs, i = [], 0
    while i < len(yls):
        y0, n = yls[i], 1
        while (
            i + n < len(yls)
            and yls[i + n] == y0 + n
            and cl[y0 + n] == cl[y0] + 2 * n
        ):
            n += 1
        runs.append((y0, n, cl[y0] - lo))
        i += n
    return runs


Y_RUNS = [_y_runs(rt0, nrt) for (rt0, nrt, _, _) in SUBS]

# ------------------------------------------------------------- bass program
_NC_CACHE = None


def _build_nc():
    import concourse.bacc as bacc
    import concourse.tile as tile
    from concourse import mybir

    nc = bacc.Bacc(None, target_bir_lowering=False, debug=False)
    f32 = mybir.dt.float32

    # CLS token already stripped host-side -> uniform (b, i) block stride.
    hs = nc.dram_tensor("hs", (B_CORE, 3136, 768), f32, kind="ExternalInput")
    out = nc.dram_tensor("out", (B_CORE, 1, 64, 64, 64), f32, kind="ExternalOutput")

    # [(b i), t, u]: 128 blocks x 196 tokens x feature
    hs_v = hs.ap().rearrange("b (i t) u -> (b i) t u", i=16)
    # [(b i), q, (y x)]: z = 4i+q; y,x merge into one contiguous dim
    out_v = out.ap().rearrange(
        "b c (i q) y x -> (b i) c q (y x)", i=16, q=4
    )

    with tile.TileContext(nc) as tc:
        with (
            tc.tile_pool(name="lp", bufs=5) as lp,
            tc.tile_pool(name="mp", bufs=3) as mp,
            tc.tile_pool(name="xp", bufs=1) as xp,
            tc.tile_pool(name="op", bufs=2) as op,
        ):
            s = 0
            for q in range(4):
                for h in range(2):
                    for sub, (rt0, nrt, yl0, nyl) in enumerate(SUBS):
                        nt = 14 * nrt  # tokens in this sub-round
                        t0 = 98 * h + 14 * rt0
                        lo = LOS[q]

                        # ---- load [128, nt, UW]; halves go to different
                        # DMA queues (HWDGE / SWDGE) and run concurrently
                        L = lp.tile([128, 56 * UW], f32, tag="L")
                        eng = nc.sync if s % 2 == 0 else nc.gpsimd
                        L3 = L[:, : nt * UW].rearrange("p (t u) -> p t u", u=UW)
                        eng.dma_start(
                            out=L3, in_=hs_v[:, t0 : t0 + nt, lo : lo + UW]
                        )

                        # ---- feature compaction -> [part, nt, 81] (ScalarE)
                        M = mp.tile([128, 56 * 81], f32, tag="M")
                        M3 = M[:, : nt * 81].rearrange("p (t j) -> p t j", j=81)
                        for (j0, n, u0) in FEAT_RUNS[q]:
                            nc.scalar.copy(
                                out=M3[:, :, j0 : j0 + n],
                                in_=L3[:, :, u0 : u0 + n],
                            )

                        # ---- x-gather -> [part, rt, d1, x] (VectorE)
                        X = xp.tile([128, 4 * 9 * 64], f32, tag="X")
                        M5 = M[:, : nt * 81].rearrange(
                            "p (rt pp d1 d2) -> p rt pp d1 d2",
                            rt=nrt, pp=14, d1=9,
                        )
                        X4 = X[:, : nrt * 9 * 64].rearrange(
                            "p (rt d1 x) -> p rt d1 x", rt=nrt, x=64
                        )
                        for (pp, x0, nx, d20) in X_RUNS:
                            nc.vector.tensor_copy(
                                out=X4[:, :, :, x0 : x0 + nx],
                                in_=M5[:, :, pp, :, d20 : d20 + 2 * nx - 1 : 2],
                            )

                        # ---- y-gather -> [part, nyl, 64] (VectorE)
                        O = op.tile([128, 18 * 64], f32, tag="O")
                        X3 = X[:, : nrt * 9 * 64].rearrange(
                            "p (cl x) -> p cl x", x=64
                        )
                        O3 = O[:, : nyl * 64].rearrange(
                            "p (yl x) -> p yl x", x=64
                        )
                        for (y0, n, c0) in Y_RUNS[sub]:
                            nc.vector.tensor_copy(
                                out=O3[:, y0 - yl0 : y0 - yl0 + n, :],
                                in_=X3[:, c0 : c0 + 2 * n - 1 : 2, :],
                            )

                        # ---- store: contiguous (y-range x 64x) per partition
                        ybase = (32 * h + yl0) * 64
                        nc.scalar.dma_start(
                            out=out_v[:, 0, q, ybase : ybase + nyl * 64],
                            in_=O[:, : nyl * 64],
                        )
                        s += 1

    nc.compile()
    return nc


def _get_nc():
    global _NC_CACHE
    if _NC_CACHE is None:
        _NC_CACHE = _build_nc()
    return _NC_CACHE


# ------------------------------------------------------------------ runner
def _in_maps(hidden_states: np.ndarray) -> list:
    hs = np.asarray(hidden_states, dtype=np.float32)
    assert hs.shape == (B_FULL, 3137, 768), hs.shape
    return [
        {"hs": np.ascontiguousarray(hs[c * B_CORE : (c + 1) * B_CORE, 1:, :])}
        for c in range(N_CORES)
    ]


def kernel(hidden_states: np.ndarray) -> np.ndarray:
    import time

    from concourse import bass_utils

    nc = _get_nc()
    in_maps = _in_maps(hidden_states)
    last_err = None
    for attempt in range(3):
        try:
            res = bass_utils.run_bass_kernel_spmd(
                nc, in_maps, core_ids=list(range(N_CORES))
            )
            return np.concatenate([r["out"] for r in res.results], axis=0)
        except Exception as e:  # transient device hiccups self-heal in ~1 min
            last_err = e
            time.sleep(45 * (attempt + 1))
    raise last_err


# revision 22
# speedup vs baseline: 1.1003x; 1.1003x over previous
"""Trainium2 Bass kernel for nn_FRAMES_VisionTransformer_28166395527587.

The reference computation (drop CLS token -> 1D nearest resize 768->729 ->
reverse-patching reshape to (144,126,126) -> 3D nearest resize to (64,64,64))
is a pure gather with compile-time-constant index maps:

    out[b, 0, z, y, x] = hs[b, 1 + 196*(z//4) + 14*r(y) + p(x),
                            f[81*d0(z) + 9*d1(y) + d2(x)]]

with  d0(z) = [0,2,4,6][z%4],          i(z) = z//4
      c(y)  = floor(63y/32) = 9*r + d1  (0, then odds 1..63, evens 64..124)
      c(x)  = floor(63x/32) = 9*p + d2  (same map)
      f[j]  = floor32(j*768/729)        (float32 floor, matching jax)

Sharding: pure data parallel, 8 batch samples per core.  The CLS token is
stripped host-side so the (sample, couple) block stride is uniform; on each
core the 128 SBUF partitions then hold the 128 (sample, couple) blocks and
every DMA spans all 128 partitions (all 16 SDMA engines).  The gather runs
as a short sequence of strided on-chip copies shared by all partitions.

Work is split into 16 sub-rounds (q, h, token-row half); the two halves of
each round load through different DMA queues (HWDGE + SWDGE) concurrently.
"""

import numpy as np

# ---------------------------------------------------------------- constants
B_FULL = 64
N_CORES = 8
B_CORE = B_FULL // N_CORES  # 8 samples per core


def _nearest_f32(out_size, in_size):
    """float32-exact emulation of the reference's jnp _nearest_idx.

    jax computes floor(arange(out) * (in/out)) in float32; at j=486 the
    product rounds to 511.999... so floor gives 511, not the exact 512."""
    ratio = np.float32(in_size / out_size)
    j = np.arange(out_size, dtype=np.int32).astype(np.float32)
    return np.floor((j * ratio).astype(np.float32)).astype(np.int64)


_f = _nearest_f32(729, 768)  # feature resize map
_c = _nearest_f32(64, 126)  # y/x resize map (= 9*r + d1)

DZ = [0, 2, 4, 6]  # d0 values for z%4
LOS = [int(_f[81 * d0]) for d0 in DZ]  # [0, 170, 341, 511]
UW = 96  # features per token: 384 B = 3x128 B aligned descriptors

# token-row split of each (q, h) round: rows 0-2 -> yl 0-13, rows 3-6 -> 14-31
SUBS = [(0, 3, 0, 14), (3, 4, 14, 18)]  # (rt0, nrt, yl0, nyl)


def _feat_runs(q):
    """Contiguous runs of the 81-feature selection for d0-slice q.

    Returns [(j0, n, u0)]: M[:, j0:j0+n] = L[:, u0:u0+n]."""
    g = _f[81 * DZ[q] + np.arange(81)] - LOS[q]
    runs, start = [], 0
    for k in range(1, 81):
        if g[k] != g[k - 1] + 1:
            runs.append((start, k - start, int(g[start])))
            start = k
    runs.append((start, 81 - start, int(g[start])))
    return runs


FEAT_RUNS = [_feat_runs(q) for q in range(4)]


def _x_runs():
    """x-gather runs: [(p, x0, nx, d20)] with d2 = d20+2k, x = x0+k."""
    runs, x = [], 0
    while x < 64:
        p, d20 = int(_c[x]) // 9, int(_c[x]) % 9
        n = 1
        while x + n < 64 and _c[x + n] == _c[x] + 2 * n and _c[x + n] // 9 == p:
            n += 1
        runs.append((p, x, n, d20))
        x += n
    return runs


X_RUNS = _x_runs()


def _y_runs(rt0, nrt):
    """y-gather runs for rows [rt0, rt0+nrt): [(yl0, n, c0)] with
    O[yl0+k] = X[c0+2k] in the sub-round's local c coordinates.

    The local map cl(yl) = c(32h+yl) - 63h is identical for both h."""
    cl = [int(_c[yl]) for yl in range(32)]  # h=0 local map: [0,1,3,...,61]
    lo, hi = 9 * rt0, 9 * (rt0 + nrt)
    yls = [yl for yl in range(32) if lo <= cl[yl] < hi]
    runs, i = [], 0
    while i < len(yls):
        y0, n = yls[i], 1
        while (
            i + n < len(yls)
            and yls[i + n] == y0 + n
            and cl[y0 + n] == cl[y0] + 2 * n
        ):
            n += 1
        runs.append((y0, n, cl[y0] - lo))
        i += n
    return runs


Y_RUNS = [_y_runs(rt0, nrt) for (rt0, nrt, _, _) in SUBS]

# ------------------------------------------------------------- bass program
_NC_CACHE = None


def _build_nc():
    import concourse.bacc as bacc
    import concourse.tile as tile
    from concourse import mybir

    nc = bacc.Bacc(None, target_bir_lowering=False, debug=False)
    f32 = mybir.dt.float32

    # CLS token already stripped host-side -> uniform (b, i) block stride.
    hs = nc.dram_tensor("hs", (B_CORE, 3136, 768), f32, kind="ExternalInput")
    out = nc.dram_tensor("out", (B_CORE, 1, 64, 64, 64), f32, kind="ExternalOutput")

    # [(b i), t, u]: 128 blocks x 196 tokens x feature
    hs_v = hs.ap().rearrange("b (i t) u -> (b i) t u", i=16)
    # [(b i), q, (y x)]: z = 4i+q; y,x merge into one contiguous dim
    out_v = out.ap().rearrange(
        "b c (i q) y x -> (b i) c q (y x)", i=16, q=4
    )

    with tile.TileContext(nc) as tc:
        with (
            tc.tile_pool(name="lp", bufs=5) as lp,
            tc.tile_pool(name="mp", bufs=3) as mp,
            tc.tile_pool(name="xp", bufs=1) as xp,
            tc.tile_pool(name="op", bufs=2) as op,
        ):
            s = 0
            for q in range(4):
                for h in range(2):
                    for sub, (rt0, nrt, yl0, nyl) in enumerate(SUBS):
                        nt = 14 * nrt  # tokens in this sub-round
                        t0 = 98 * h + 14 * rt0
                        lo = LOS[q]

                        # ---- load [128, nt, UW]; halves go to different
                        # DMA queues (HWDGE / SWDGE) and run concurrently
                        # payload 88 floats (352 B) per token, dst stride 96
                        # (384 B) keeps every SBUF write 128 B aligned
                        L = lp.tile([128, 56 * UW], f32, tag="L")
                        eng = nc.sync if s % 2 == 0 else nc.gpsimd
                        L3 = L[:, : nt * UW].rearrange("p (t u) -> p t u", u=UW)
                        eng.dma_start(
                            out=L3[:, :, :88], in_=hs_v[:, t0 : t0 + nt, lo : lo + 88]
                        )

                        # ---- feature compaction -> [part, nt, 81] (ScalarE)
                        M = mp.tile([128, 56 * 81], f32, tag="M")
                        M3 = M[:, : nt * 81].rearrange("p (t j) -> p t j", j=81)
                        for (j0, n, u0) in FEAT_RUNS[q]:
                            nc.scalar.copy(
                                out=M3[:, :, j0 : j0 + n],
                                in_=L3[:, :, u0 : u0 + n],
                            )

                        # ---- x-gather -> [part, rt, d1, x] (VectorE)
                        X = xp.tile([128, 4 * 9 * 64], f32, tag="X")
                        M5 = M[:, : nt * 81].rearrange(
                            "p (rt pp d1 d2) -> p rt pp d1 d2",
                            rt=nrt, pp=14, d1=9,
                        )
                        X4 = X[:, : nrt * 9 * 64].rearrange(
                            "p (rt d1 x) -> p rt d1 x", rt=nrt, x=64
                        )
                        for (pp, x0, nx, d20) in X_RUNS:
                            nc.vector.tensor_copy(
                                out=X4[:, :, :, x0 : x0 + nx],
                                in_=M5[:, :, pp, :, d20 : d20 + 2 * nx - 1 : 2],
                            )

                        # ---- y-gather -> [part, nyl, 64] (VectorE)
                        O = op.tile([128, 18 * 64], f32, tag="O")
                        X3 = X[:, : nrt * 9 * 64].rearrange(
                            "p (cl x) -> p cl x", x=64
                        )
                        O3 = O[:, : nyl * 64].rearrange(
                            "p (yl x) -> p yl x", x=64
                        )
                        for (y0, n, c0) in Y_RUNS[sub]:
                            nc.vector.tensor_copy(
                                out=O3[:, y0 - yl0 : y0 - yl0 + n, :],
                                in_=X3[:, c0 : c0 + 2 * n - 1 : 2, :],
                            )

                        # ---- store: contiguous (y-range x 64x) per partition
                        ybase = (32 * h + yl0) * 64
                        nc.scalar.dma_start(
                            out=out_v[:, 0, q, ybase : ybase + nyl * 64],
                            in_=O[:, : nyl * 64],
                        )
                        s += 1

    nc.compile()
    return nc


def _get_nc():
    global _NC_CACHE
    if _NC_CACHE is None:
        _NC_CACHE = _build_nc()
    return _NC_CACHE


# ------------------------------------------------------------------ runner
def _in_maps(hidden_states: np.ndarray) -> list:
    hs = np.asarray(hidden_states, dtype=np.float32)
    assert hs.shape == (B_FULL, 3137, 768), hs.shape
    return [
        {"hs": np.ascontiguousarray(hs[c * B_CORE : (c + 1) * B_CORE, 1:, :])}
        for c in range(N_CORES)
    ]


def kernel(hidden_states: np.ndarray) -> np.ndarray:
    import time

    from concourse import bass_utils

    nc = _get_nc()
    in_maps = _in_maps(hidden_states)
    last_err = None
    for attempt in range(3):
        try:
            res = bass_utils.run_bass_kernel_spmd(
                nc, in_maps, core_ids=list(range(N_CORES))
            )
            return np.concatenate([r["out"] for r in res.results], axis=0)
        except Exception as e:  # transient device hiccups self-heal in ~1 min
            last_err = e
            time.sleep(45 * (attempt + 1))
    raise last_err
